# revision 18
# baseline (speedup 1.0000x reference)
"""Trainium2 Bass kernel for nn_BiAffineForward (bilinear relation scorer).

Data-parallel over N=128 across 8 NeuronCores (16 batches/core). Heavy
matmuls in fp8e4m3 (scaled by powers of 2); index structures become
host-built gather lists / one-hot matmul patterns.

Per-core pipeline:
  WU  PE warmup matmuls during the initial DMA window (HAM un-throttle)
  P1  dedup'd span-row gather (fp8 x16, row PAIRS per descriptor) +
      span-mean via TRANSPOSED pattern matmuls -> spansT [128, 6, 384]
      directly (no SBUF transpose pass)
  P2  two-layer MLP in bf16 on packed 384 span cols
  P3  mention gather+mask via one-hot matmul -> hg (fp8, x16) / tg
      (bf16, x2^-16); d-remainder rows in zero-padded K=32 tiles
  P4  bilinear stage 1 per r in fp8 (weights x256): e-rem groups first
      (so the hwf flatten DMAs overlap the main loop), then per-r mains
  P5  stage 2 col-tiled: 4 batches concurrently per matmul via
      tile_position=(0,32g); psum evac fused with log-count add -> lts
  P6  exp (ACT) -> segmented reduce (DVE) -> partition-sum matmul ->
      one batched Ln + bias -> single output DMA

logsumexp uses no max-subtraction: logits are O(1e-2) by construction and
masked pairs carry log-count -88 -> exp()==0.
"""

import os
import sys

import numpy as np

sys.path.insert(0, "/opt/trn_rl_repo")

N, L, D = 128, 512, 768
M, H, T, HT = 24, 10, 10, 100
FF, R = 140, 97
NCORES = 8
NL = N // NCORES          # 16 batches per core
NH = NL * H               # 160 (n,i) columns per core
NI = R * H                # 970
GQ = 4                    # gather groups (4 batches each)
NG = (R + 3) // 4         # 25 e-remainder groups (4 r's each)
SC_SENT = 16.0            # sent fp8 scale
SC_W = 256.0              # bilinear weight fp8 scale
SC_TG = 2.0 ** -16        # tg evac scale (undoes 16*16*256 = 2^16)

_cache = {}


def _build(maxcnt):
    assert all(int(c) <= 256 for c in maxcnt), f"quad slots exceed 256: {maxcnt}"
    key = tuple(int(x) for x in maxcnt)
    if key in _cache:
        return _cache[key]
    from contextlib import ExitStack

    import concourse.bacc as bacc
    import concourse.bass as bass
    import concourse.mybir as mybir
    import concourse.tile as tile

    f32 = mybir.dt.float32
    bf16 = mybir.dt.bfloat16
    fp8 = mybir.dt.float8e4
    i16 = mybir.dt.int16
    AF = mybir.ActivationFunctionType

    nc = bacc.Bacc(
        "TRN2",
        target_bir_lowering=False,
        debug=False,
        num_devices=NCORES,
        num_swdge_queues=4,
    )

    def din(name, shape, dt=bf16):
        return nc.dram_tensor(name, list(shape), dt, kind="ExternalInput").ap()

    sent = din("sent", (NL * L - 3, 4 * D), fp8)   # overlapping row quads
    gidx = din("gidx", (128, GQ * 16), i16)
    gpat = din("gpat", (128, GQ, 4, 2, 96), fp8)
    w1h = din("w1h", (D, FF))
    w2h = din("w2h", (FF, FF))
    w1t = din("w1t", (D, FF))
    w2t = din("w2t", (FF, FF))
    gbh = din("gbh", (384, NH))
    gbt = din("gbt", (384, NH))
    wt0 = din("wt0", (128, R * 128), fp8)          # main d<128, e<128, r-major
    wt1z = din("wt1z", (32, R * 128), fp8)         # d-rem rows, zero-padded
    wr0 = din("wr0", (128, NG * 128), fp8)         # e-rem, d<128, 4r/block
    wr1z = din("wr1z", (32, NG * 128), fp8)        # e-rem d-rem, zero-padded
    lcnt = din("lcnt", (128, 4, NI))               # log-counts, bf16
    spat = din("spat", (128, 4), f32)
    bvecr = din("bvecr", (4, 4 * R), f32)
    outp = nc.dram_tensor("out", [NL, R], f32, kind="ExternalOutput").ap()
    dbg = os.environ.get("KDBG", "0") == "1"
    if dbg:
        d_spansT = nc.dram_tensor("d_spansT", [128, 6 * 384], mybir.dt.bfloat16,
                                  kind="ExternalOutput").ap()
        d_hg = nc.dram_tensor("d_hg", [128, 2 * NH], mybir.dt.float8e4,
                              kind="ExternalOutput").ap()
        d_tg = nc.dram_tensor("d_tg", [128, NH + NL * 32], mybir.dt.bfloat16,
                              kind="ExternalOutput").ap()
        d_hwt = nc.dram_tensor("d_hwt", [128, NL * NI], mybir.dt.bfloat16,
                               kind="ExternalOutput").ap()
        d_hwf = nc.dram_tensor("d_hwf", [12, R * NH], mybir.dt.bfloat16,
                               kind="ExternalOutput").ap()
        d_lt = nc.dram_tensor("d_lt", [128, 4 * NI], f32,
                              kind="ExternalOutput").ap()

    with tile.TileContext(nc) as tc, ExitStack() as ctx:
        const = ctx.enter_context(tc.tile_pool(name="const", bufs=1))
        persist = ctx.enter_context(tc.tile_pool(name="persist", bufs=1))

        # ---- constants into SBUF
        gidx_sb = const.tile([128, GQ * 16], i16)
        nc.sync.dma_start(gidx_sb[:], gidx[:, :])
        gpat_sb = const.tile([128, GQ, 4, 2, 96], fp8)
        nc.sync.dma_start(gpat_sb[:], gpat[:, :, :, :, :])
        w1h_sb = const.tile([128, 6, FF], bf16)
        nc.sync.dma_start(w1h_sb[:], w1h.rearrange("(c p) f -> p c f", p=128))
        w1t_sb = const.tile([128, 6, FF], bf16)
        nc.sync.dma_start(w1t_sb[:], w1t.rearrange("(c p) f -> p c f", p=128))
        w2h_sb = const.tile([128, 2, FF], bf16)
        nc.sync.dma_start(w2h_sb[:, 0, :], w2h[0:128, :])
        nc.sync.dma_start(w2h_sb[0:12, 1, :], w2h[128:FF, :])
        w2t_sb = const.tile([128, 2, FF], bf16)
        nc.sync.dma_start(w2t_sb[:, 0, :], w2t[0:128, :])
        nc.sync.dma_start(w2t_sb[0:12, 1, :], w2t[128:FF, :])
        gbh_sb = const.tile([128, 3, NH], bf16)
        nc.sync.dma_start(gbh_sb[:], gbh.rearrange("(c p) g -> p c g", p=128))
        gbt_sb = const.tile([128, 3, NH], bf16)
        nc.sync.dma_start(gbt_sb[:], gbt.rearrange("(c p) g -> p c g", p=128))
        lcnt_sb = const.tile([128, 4, NI], bf16)
        spat_sb = const.tile([128, 4], f32)
        nc.sync.dma_start(spat_sb[:], spat[:, :])
        bvec_sb = const.tile([4, 4 * R], f32)
        nc.sync.dma_start(bvec_sb[:], bvecr[:, :])

        # ---- persistent tensors
        wt0_sb = persist.tile([128, R * 128], fp8)
        wt1p = persist.tile([32, R * 128], fp8)
        wr0_sb = persist.tile([128, NG * 128], fp8)
        wr1p = persist.tile([32, NG * 128], fp8)
        hg0 = persist.tile([128, NH], fp8)
        hg1p = persist.tile([32, NH], fp8)
        nc.vector.memset(hg1p[:], 0.0)
        tg0 = persist.tile([128, NH], bf16)
        tg1p = persist.tile([12, NL * 32], bf16)
        nc.vector.memset(tg1p[:], 0.0)
        hwt = persist.tile([128, NL * NI], bf16)
        hwv = hwt.rearrange("p (n r i) -> p n r i", n=NL, r=R)
        hwrem = persist.tile([128, NG * NH], bf16)
        hwf = persist.tile([12, R * NH], bf16)
        spansT = persist.tile([128, 6, 384], bf16)
        lts = [persist.tile([128, NI], f32, name=f"lt{t}") for t in range(4)]
        junk = persist.tile([128, 512], fp8)
        nc.gpsimd.memset(junk[:], 0.0)

        # =========================== Phase A ===========================
        with tc.tile_pool(name="pa_str", bufs=4) as pstr, \
             tc.tile_pool(name="pa_ps", bufs=2, space="PSUM") as pps, \
             tc.tile_pool(name="pa_ps2", bufs=2, space="PSUM") as pps2, \
             tc.tile_pool(name="pa_ps3", bufs=1, space="PSUM") as pps3, \
             tc.tile_pool(name="pa_wu", bufs=1, space="PSUM") as pwu, \
             tc.tile_pool(name="pa_sb", bufs=1) as pa:

            # dummy gather at t~0: triggers the Q7 dma_gather ucode IRAM
            # load (~6us) during the initial DMA window
            widx = pa.tile([128, 8], i16)
            nc.vector.memset(widx[:], 0)
            wgt = pa.tile([128, 256], fp8)
            nc.gpsimd.dma_gather(
                out_ap=wgt[:, :].rearrange("p (c e) -> p c e", c=1),
                in_ap=sent[:, 0:256],
                idxs_ap=widx[:, :],
                num_idxs=128,
                num_idxs_reg=128,
                elem_size=256,
                elem_step=4 * D,
                queue_num=0,
            )

            # PE warmup during the initial DMA window
            wups = pwu.tile([128, 512], f32)
            for _ in range(40):
                nc.tensor.matmul(wups[:, 0:128], lhsT=junk[:, 0:128],
                                 rhs=junk[:, 0:128],
                                 start=True, stop=True)

            # P1: per-q gather + transposed span-mean matmuls (quad rows)
            for q in range(GQ):
                ncc = (int(maxcnt[q]) + 127) // 128  # live gather chunks
                gt = pstr.tile([128, 2, 4 * D], fp8, tag="gt")
                nc.gpsimd.dma_gather(
                    out_ap=gt[:, :, :],
                    in_ap=sent[:, :],
                    idxs_ap=gidx_sb[:, q * 16:(q + 1) * 16],
                    num_idxs=256,
                    num_idxs_reg=ncc * 128,
                    elem_size=4 * D,
                    queue_num=q % 4,
                )
                for dpair in range(3):
                    ps1 = pps.tile([128, 512], f32, tag="ps1")
                    for dc in range(2):
                        kc = 2 * dpair + dc
                        for gc in range(ncc):
                            for ab in range(4):
                                nc.tensor.matmul(
                                    ps1[:, dc * 96:dc * 96 + 96],
                                    lhsT=gt[:, gc,
                                            ab * D + kc * 128:
                                            ab * D + kc * 128 + 128],
                                    rhs=gpat_sb[:, q, ab, gc, :],
                                    start=(gc == 0 and ab == 0),
                                    stop=(gc == ncc - 1 and ab == 3),
                                )
                    e0 = nc.vector.tensor_copy if dpair % 2 == 0 \
                        else nc.scalar.copy
                    e1 = nc.scalar.copy if dpair % 2 == 0 \
                        else nc.vector.tensor_copy
                    e0(spansT[:, 2 * dpair, q * 96:q * 96 + 96], ps1[:, 0:96])
                    e1(spansT[:, 2 * dpair + 1, q * 96:q * 96 + 96],
                       ps1[:, 96:192])

            # big weight loads via SWDGE *after* the gathers in gpsimd
            # program order: gathers get full DMA bandwidth first, weights
            # stream in underneath Phase A compute (needed ~NG loop onward)
            nc.gpsimd.dma_start(wr0_sb[:], wr0[:, :])
            nc.gpsimd.dma_start(wr1p[:], wr1z[:, :])
            nc.gpsimd.dma_start(wt0_sb[:], wt0[:, :])
            nc.gpsimd.dma_start(wt1p[:], wt1z[:, :])
            nc.gpsimd.dma_start(lcnt_sb[:], lcnt[:, :, :])

            # P2 + P3 per side (packed 384 span cols)
            for w1sb, w2sb, gbsb, side in (
                (w1h_sb, w2h_sb, gbh_sb, "h"),
                (w1t_sb, w2t_sb, gbt_sb, "t"),
            ):
                ps2a = pps2.tile([128, 512], f32, tag="ps2a")
                ps2b = pps2.tile([12, 512], f32, tag="ps2b")
                for kc in range(6):
                    nc.tensor.matmul(
                        ps2a[:, 0:384], lhsT=w1sb[:, kc, 0:128],
                        rhs=spansT[:, kc, :], start=(kc == 0), stop=(kc == 5),
                    )
                for kc in range(6):
                    nc.tensor.matmul(
                        ps2b[:, 0:384], lhsT=w1sb[:, kc, 128:FF],
                        rhs=spansT[:, kc, :], start=(kc == 0), stop=(kc == 5),
                    )
                a0 = pa.tile([128, 384], bf16, tag="a0")
                a1 = pa.tile([12, 384], bf16, tag="a1")
                nc.vector.tensor_relu(a0[:], ps2a[:, 0:384])
                nc.vector.tensor_relu(a1[:], ps2b[:, 0:384])
                b2 = pa.tile([128, 3, FF], bf16, tag="b2")
                for sc in range(3):
                    ps2c = pps3.tile([128, 512], f32, tag="ps2c")
                    sl = slice(sc * 128, (sc + 1) * 128)
                    nc.tensor.matmul(
                        ps2c[:, 0:FF], lhsT=a0[:, sl], rhs=w2sb[:, 0, :],
                        start=True, stop=False,
                    )
                    nc.tensor.matmul(
                        ps2c[:, 0:FF], lhsT=a1[:, sl], rhs=w2sb[0:12, 1, :],
                        start=False, stop=True,
                    )
                    if sc % 2 == 0:
                        nc.vector.tensor_copy(b2[:, sc, :], ps2c[:, 0:FF])
                    else:
                        nc.scalar.copy(b2[:, sc, :], ps2c[:, 0:FF])
                ps3c = pps2.tile([128, 512], f32, tag="ps2a")
                ps3 = ps3c[:, 0:NH]
                ps3r = ps3c[0:12, 256:256 + NH]
                for sc in range(3):
                    nc.tensor.matmul(
                        ps3[:, :], lhsT=b2[:, sc, 0:128], rhs=gbsb[:, sc, :],
                        start=(sc == 0), stop=(sc == 2),
                    )
                for sc in range(3):
                    # own group; start's bank-wide has_written clear is
                    # harmless (ps3 is complete, only read afterwards)
                    nc.tensor.matmul(
                        ps3r[:, :], lhsT=b2[:, sc, 128:FF],
                        rhs=gbsb[:, sc, :],
                        start=(sc == 0), stop=(sc == 2),
                    )
                if side == "h":
                    nc.scalar.activation(hg0[:], ps3[:, :], AF.Copy)
                    nc.vector.tensor_copy(hg1p[0:12, :], ps3r[:, :])
                else:
                    nc.scalar.activation(tg0[:], ps3[:, :], AF.Copy,
                                         scale=SC_TG)
                    nc.vector.tensor_scalar_mul(
                        tg1p.rearrange("p (n j) -> p n j", n=NL)[:, :, 0:10],
                        ps3r.rearrange("p (n i) -> p n i", n=NL)[:, :, :],
                        SC_TG)

        # =========================== Phase B ===========================
        QUARTERS = ((0, 25), (25, 25), (50, 25), (75, 22))

        def p5_tile(p5m, p6sb, t, qr0, qw):
            """One stage-2 tile: batches 4t..4t+3 (col-tiled), r-range
            [qr0, qr0+qw). Evac fused with log-count add into lts[t]."""
            FRq = qw * H
            c0 = qr0 * H
            ps5 = p5m.tile([128, 512], f32, tag="ps5")
            # K=1 zero matmul: opens the bank (bank-wide has_written clear)
            # and writes zeros everywhere (sim + HW correct)
            nc.tensor.matmul(
                ps5[:, 0:FRq], lhsT=junk[0:1, 0:128],
                rhs=junk[0:1, 0:FRq], start=True, stop=False,
            )
            for g in range(4):
                n = 4 * t + g
                nc.tensor.matmul(
                    ps5[32 * g:32 * g + 10, 0:FRq],
                    lhsT=tg0[:, n * H:(n + 1) * H],
                    rhs=hwv[:, n, qr0:qr0 + qw, :],
                    start=False, stop=False,
                    tile_position=(0, 32 * g),
                    skip_group_check=True,
                )
            for g in range(4):
                n = 4 * t + g
                nc.tensor.matmul(
                    ps5[32 * g:32 * g + 32, 0:FRq],
                    lhsT=tg1p[:, n * 32:(n + 1) * 32],
                    rhs=hwfv[:, qr0:qr0 + qw, n, :],
                    start=False, stop=False,
                    tile_position=(0, 32 * g),
                    skip_group_check=True,
                )
            # closing K=1 zero matmul: +0 everywhere, stop for the full bank
            nc.tensor.matmul(
                ps5[:, 0:FRq], lhsT=junk[0:1, 0:128],
                rhs=junk[0:1, 0:FRq], start=False, stop=True,
            )
            nc.vector.tensor_add(
                lts[t][:, c0:c0 + FRq],
                ps5[:, 0:FRq],
                lcnt_sb[:, t, c0:c0 + FRq],
            )

        hwfv = hwf.rearrange("p (r n i) -> p r n i", r=R, n=NL)
        with tc.tile_pool(name="pb_ps", bufs=4, space="PSUM") as bps:
            with tc.tile_pool(name="pb_psB", bufs=2, space="PSUM") as bpsB:
                # e-remainder groups FIRST (flatten DMAs overlap mains)
                for g in range(NG):
                    pB = bpsB.tile([128, 512], f32, tag="pB")
                    nc.tensor.matmul(
                        pB[:, 0:NH], lhsT=wr0_sb[:, g * 128:(g + 1) * 128],
                        rhs=hg0[:], start=True, stop=False,
                    )
                    nc.tensor.matmul(
                        pB[:, 0:NH], lhsT=wr1p[:, g * 128:(g + 1) * 128],
                        rhs=hg1p[:], start=False, stop=True,
                    )
                    if g % 2 == 0:
                        nc.vector.tensor_copy(
                            hwrem[:, g * NH:(g + 1) * NH], pB[:, 0:NH])
                    else:
                        nc.scalar.copy(
                            hwrem[:, g * NH:(g + 1) * NH], pB[:, 0:NH])
            # flatten hwrem strips -> hwf [12, (r, n, i)] via DMA
            hfm = hwf[:, 0:96 * NH].rearrange(
                "p (g j x) -> p g j x", j=4, x=NH)
            for jj in range(4):
                nc.sync.dma_start(
                    hfm[:, :, jj, :],
                    hwrem[32 * jj:32 * jj + 12, 0:24 * NH].rearrange(
                        "p (g x) -> p g x", x=NH),
                )
            nc.sync.dma_start(
                hwf[:, 96 * NH:97 * NH], hwrem[0:12, 24 * NH:25 * NH])

            # mains in r-quarters; P5 tiles of quarter b interleave into
            # the mains of quarter b+1 (keeps PE-array duty high for HAM)
            with tc.tile_pool(name="p5m", bufs=2, space="PSUM") as p5m, \
                 tc.tile_pool(name="p6ps", bufs=1, space="PSUM") as p6ps, \
                 tc.tile_pool(name="p6sb", bufs=1) as p6:
                ps6all = p6ps.tile([4, 512], f32)
                pend = []

                def emit_triple(r0, w):
                    pA = bps.tile([128, 512], f32, tag="pA")
                    for jj in range(w):
                        rr = r0 + jj
                        nc.tensor.matmul(
                            pA[:, jj * NH:(jj + 1) * NH],
                            lhsT=wt0_sb[:, rr * 128:rr * 128 + 128],
                            rhs=hg0[:], start=True, stop=False,
                        )
                        nc.tensor.matmul(
                            pA[:, jj * NH:(jj + 1) * NH],
                            lhsT=wt1p[:, rr * 128:rr * 128 + 128],
                            rhs=hg1p[:], start=False, stop=True,
                        )
                    src_v = pA[:, 0:w * NH].rearrange(
                        "p (r n i) -> p n r i", r=w, n=NL)
                    if (r0 // 3) % 2 == 0:
                        nc.vector.tensor_copy(
                            hwv[:, :, r0:r0 + w, :], src_v)
                    else:
                        nc.scalar.copy(hwv[:, :, r0:r0 + w, :], src_v)

                for b, (qr0, qw) in enumerate(QUARTERS):
                    r = qr0
                    while r < qr0 + qw:
                        w = min(3, qr0 + qw - r)
                        emit_triple(r, w)
                        r += w
                        if pend and (r // 3) % 2 == 0:
                            pend.pop(0)()
                    for t in range(4):
                        pend.append(
                            (lambda t=t, qr0=qr0, qw=qw:
                             p5_tile(p5m, p6, t, qr0, qw)))
                # drain remaining P5 tiles (incl. all of quarter 3)
                for fn in pend:
                    fn()
                # P6 heads: exp + segmented reduce + partition-sum matmul
                for t in range(4):
                    et = p6.tile([128, NI], f32, tag="et", bufs=2)
                    nc.scalar.activation(et[:], lts[t][:], AF.Exp)
                    s1 = p6.tile([128, R], f32, tag="s1", bufs=2)
                    nc.vector.tensor_reduce(
                        s1[:],
                        et.rearrange("p (r i) -> p r i", r=R),
                        axis=mybir.AxisListType.X,
                        op=mybir.AluOpType.add,
                    )
                    nc.tensor.matmul(
                        ps6all[:, t * R:(t + 1) * R], lhsT=spat_sb[:],
                        rhs=s1[:],
                        start=(t == 0), stop=False,
                    )
                # closing zero matmul over the whole ps6 bank
                nc.tensor.matmul(
                    ps6all[:, 0:4 * R], lhsT=junk[0:1, 0:4],
                    rhs=junk[0:1, 0:4 * R], start=False, stop=True,
                )
                # P6 tail: ONE batched Ln + bias + output DMAs
                res = p6.tile([4, 4 * R], f32, tag="res")
                nc.scalar.activation(res[:], ps6all[:, 0:4 * R], AF.Ln)
                res2 = p6.tile([4, 4 * R], f32, tag="res2")
                nc.vector.tensor_add(res2[:], res[:], bvec_sb[:])
                for t in range(4):
                    nc.sync.dma_start(outp[4 * t:4 * t + 4, :],
                                      res2[:, t * R:(t + 1) * R])

    nc.compile()
    _cache[key] = nc
    return nc


def _pack_quads(rset, max_start):
    """rset: {row: [(col, w), ...]} (rows core-local). Returns list of
    (quad_start_row, [memb_0, memb_1, memb_2, memb_3]) covering every union
    row exactly once; a quad reads rows s..s+3 with s <= max_start."""
    rows = sorted(rset)
    slots = []

    def flush(run):
        t = 0
        while t < len(run):
            s = min(run[t], max_start)
            membs = [[], [], [], []]
            while t < len(run) and run[t] - s < 4:
                membs[run[t] - s] = rset[run[t]]
                t += 1
            slots.append((s, membs))

    run = []
    for r in rows:
        if run and r == run[-1] + 1:
            run.append(r)
        else:
            if run:
                flush(run)
            run = [r]
    if run:
        flush(run)
    return slots


def _host_prep(inputs):
    """Shard + build index-derived matrices. Returns (in_maps, maxcnt)."""
    import ml_dtypes

    nbf = ml_dtypes.bfloat16
    nf8 = ml_dtypes.float8_e4m3
    sent_f = np.asarray(inputs["sentence_repr"], np.float32)
    spans = np.asarray(inputs["entity_span_indices"]).astype(np.int64)
    hidx = np.asarray(inputs["head_mentions_indices"]).astype(np.int64)
    hmask = np.asarray(inputs["head_mentions_indices_mask"]).astype(np.int64)
    tidx_i = np.asarray(inputs["tail_mentions_indices"]).astype(np.int64)
    tmask = np.asarray(inputs["tail_mentions_indices_mask"]).astype(np.int64)
    hti = np.asarray(inputs["ht_comb_indices"]).astype(np.int64)
    htm = np.asarray(inputs["ht_comb_mask"]).astype(np.int64)

    s_, e_ = spans[..., 0], spans[..., 1]

    # --- per (core, group): dedup'd union rows -> pair slots
    slots_cq = [[None] * GQ for _ in range(NCORES)]
    counts = np.zeros((NCORES, GQ), np.int64)
    for c in range(NCORES):
        for q in range(GQ):
            allslots = []
            for g in range(4):
                nn = c * NL + 4 * q + g
                rset = {}
                for m in range(M):
                    s0, e0 = int(s_[nn, m]), int(e_[nn, m])
                    w = 1.0 / (e0 - s0 + 1)
                    for r in range(s0, e0 + 1):
                        rset.setdefault((4 * q + g) * L + r, []).append(
                            (24 * g + m, w)
                        )
                allslots.extend(_pack_quads(rset, NL * L - 4))
            counts[c, q] = len(allslots)
            slots_cq[c][q] = allslots
    maxcnt = counts.max(axis=0)

    spat = np.zeros((128, 4), np.float32)
    for g in range(4):
        spat[32 * g:32 * g + T, g] = 1.0
    bili_b = np.asarray(inputs["bili_b"], np.float32)
    bvecr = np.zeros((4, 4 * R), np.float32)
    for t in range(4):
        bvecr[:, t * R:(t + 1) * R] = bili_b[None, :]
    bili_W = np.asarray(inputs["bili_W"], np.float32) * SC_W
    # wt0: [d<128, (r, e<128)]
    wt0 = np.ascontiguousarray(
        bili_W[:, 0:128, 0:128].transpose(1, 0, 2).reshape(128, R * 128)
    ).astype(nf8)
    # wt1z: [d-rem 12 (+20 zero pad rows), (r, e<128)]
    wt1z = np.zeros((32, R * 128), np.float32)
    wt1z[0:12, :] = bili_W[:, 128:FF, 0:128].transpose(1, 0, 2).reshape(
        12, R * 128)
    wt1z = wt1z.astype(nf8)
    # wr0 / wr1z: e-rem packed, 4 r's per 128-col block at 32-col strips
    wr0 = np.zeros((128, NG * 128), np.float32)
    wr1z = np.zeros((32, NG * 128), np.float32)
    for g in range(NG):
        for jj in range(min(4, R - 4 * g)):
            r = 4 * g + jj
            cs = g * 128 + 32 * jj
            wr0[:, cs:cs + 12] = bili_W[r, 0:128, 128:FF]
            wr1z[0:12, cs:cs + 12] = bili_W[r, 128:FF, 128:FF]
    wr0 = wr0.astype(nf8)
    wr1z = wr1z.astype(nf8)

    shared = dict(
        w1h=np.asarray(inputs["W1h"], np.float32).astype(nbf),
        w2h=np.asarray(inputs["W2h"], np.float32).astype(nbf),
        w1t=np.asarray(inputs["W1t"], np.float32).astype(nbf),
        w2t=np.asarray(inputs["W2t"], np.float32).astype(nbf),
        wt0=wt0, wt1z=wt1z, wr0=wr0, wr1z=wr1z,
        spat=spat, bvecr=bvecr,
    )

    in_maps = []
    for c in range(NCORES):
        ns = slice(c * NL, (c + 1) * NL)
        # overlapping row-pair view of this core's sentence tensor (fp8, x16)
        sc_flat = np.ascontiguousarray(
            sent_f[ns].reshape(NL * L * D) * SC_SENT
        ).astype(nf8)
        sent_p = np.lib.stride_tricks.as_strided(
            sc_flat, shape=(NL * L - 3, 4 * D), strides=(D, 1)
        )
        gidx16 = -np.ones((16, GQ * 16), np.int16)
        gpat = np.zeros((128, GQ, 4, 2, 96), np.float32)
        for q in range(GQ):
            slots = list(slots_cq[c][q])
            cpad = ((int(maxcnt[q]) + 127) // 128) * 128
            while len(slots) < cpad:
                slots.append((0, [[], [], [], []]))
            for i, (srow, membs) in enumerate(slots):
                gidx16[i % 16, q * 16 + i // 16] = srow
                for ab in range(4):
                    for col, w in membs[ab]:
                        gpat[i % 128, q, ab, i // 128, col] += w
        gidx = np.tile(gidx16, (8, 1))

        gbh = np.zeros((384, NH), np.float32)
        gbt = np.zeros((384, NH), np.float32)
        for n in range(NL):
            for i in range(H):
                gbh[24 * n + hidx[ns][n, i], n * H + i] = float(hmask[ns][n, i])
                gbt[24 * n + tidx_i[ns][n, i], n * T + i] = float(
                    tmask[ns][n, i])
        # log-count matrix [128 (32g+j), 4 (t), NI (r,i)], bf16
        cnt = np.zeros((4, 128, H), np.float32)
        for n in range(NL):
            t2, g = divmod(n, 4)
            for p in range(HT):
                if htm[ns][n, p]:
                    i, j = hti[ns][n, p, 0], hti[ns][n, p, 1]
                    cnt[t2, 32 * g + j, i] += 1.0
        lc10 = np.where(cnt > 0, np.log(np.maximum(cnt, 1e-30)), -88.0)
        lcnt = np.tile(lc10.transpose(1, 0, 2), (1, 1, R)).reshape(128, 4, NI)
        im = dict(
            sent=np.ascontiguousarray(sent_p),
            gidx=gidx, gpat=gpat.astype(nf8),
            gbh=gbh.astype(nbf), gbt=gbt.astype(nbf),
            lcnt=lcnt.astype(nbf),
        )
        im.update(shared)
        in_maps.append(im)
    return in_maps, maxcnt


def kernel(**inputs) -> np.ndarray:
    from concourse.bass_utils import run_bass_kernel_spmd

    in_maps, maxcnt = _host_prep(inputs)
    nc = _build(maxcnt)
    res = run_bass_kernel_spmd(nc, in_maps, list(range(NCORES)))
    out = np.concatenate([res.results[c]["out"] for c in range(NCORES)], axis=0)
    return out.astype(np.float32)
